# revision 1
# baseline (speedup 1.0000x reference)
"""Trainium2 Bass kernel for nn_F0Collisions (Chang-Cooper implicit collision step).

Approach: each row's tridiagonal system depends on the row only through
s = 2*beta*dv (beta from two moments of f0x), and s spans a narrow range. The
Thomas-solve scan coefficients
    At_j = -l_j / t_{j-1}   (forward:  z_j   = At_j z_{j-1} + f_j)
    ch_j = -u_j / t_{j+1}   (backward: chi_j = ch_j chi_{j+1} + z_j)
    it_j =  1 / t_j         (final:    x_j   = it_j * chi_j)
(t = LU pivots) are analytic in s; a degree-3 Chebyshev fit per j gives ~2e-5
end-to-end error (the f32 reference itself is ~2e-4 from f64).

Per 128-row block on chip:
  2 DVE tensor_tensor_reduce -> moments -> sigma -> powers [128,4]
  PE transpose -> lhsT [4,128] (consumed as tf32/f32r)
  3 polys x 2 halves x 2 products (coeffs split hi/lo for tf32 accuracy)
    = 12 accumulating f32r matmuls -> PSUM [128,1024] each
  DVE scan1 (fwd), DVE scan2 (bwd via reversed APs), Pool multiply x = it*chi.

8 cores, data-parallel over rows: 512 rows/core.
"""
import numpy as np

NX, NV = 4096, 1024
VMAX, NUEE = 8.0, 1.0
DV = VMAX / NV
V = (np.arange(NV, dtype=np.float64) + 0.5) * DV
V_EDGE = np.arange(NV + 1, dtype=np.float64) * DV
N_CORES = 8
ROWS = NX // N_CORES          # 512 rows per core
NBLK = ROWS // 128            # 4 blocks of 128 rows
DEG = 3                       # Chebyshev degree in sigma

_prog_cache = {}

# schedule/shape knobs (tuned via TimelineSim)
CFG = {
    "bufs": 2,          # SBUF pool depth
    "scan_split": False, # half-width PSUM poly tiles + chained half scans
    "xmul": "pool",     # "pool" (ACT copy + gpsimd mul) or "dve" (read PSUM)
    "warmup": 0,        # dummy PE transposes to ramp the PE clock
    "n4": "dve",        # "dve" (stt chain) or "pool_act" (Pool f*v4 + ACT accum)
}


def _tf32_rne(x):
    xi = np.asarray(x, np.float32).view(np.uint32)
    r = (xi.astype(np.uint64) + 0x1000 + ((xi >> 13) & 1)).astype(np.uint64)
    return (r & np.uint64(0xFFFFE000)).astype(np.uint32).view(np.float32)


def _cc_delta(w):
    small = np.abs(w) < 1e-8
    ws = np.where(small, 1.0, w)
    return np.where(small, 0.5, 1.0 / ws - 1.0 / np.expm1(ws))


def _scan_coeffs_of_s(s, dt_val):
    """Exact At, ch, it for scalar s = 2*beta*DV (float64)."""
    ve = V_EDGE
    rD = 1.0 / s                       # D/DV = 1/(2 beta DV)
    delta = _cc_delta(s * ve)
    a = ve * delta - rD
    b = ve * (1.0 - delta) + rD
    a[0] = b[0] = a[NV] = b[NV] = 0.0
    coef = dt_val * (NUEE / V**2) / DV
    l = coef * a[:-1]
    d = 1.0 - coef * (a[1:] - b[:-1])
    u = -coef * b[1:]
    t = np.empty(NV)
    t[0] = d[0]
    for j in range(1, NV):
        t[j] = d[j] - l[j] * u[j - 1] / t[j - 1]
    At = np.zeros(NV); At[1:] = -l[1:] / t[:-1]
    it = 1.0 / t
    ch = np.zeros(NV); ch[:-1] = -u[:-1] / t[1:]
    return At, ch, it


def _fit_pc(dt_val, lo, hi):
    """Degree-DEG fit in sigma=(s-c0)/h for At, ch, it.
    Returns pc [(3*(DEG+1)), NV] f32 (row 4p+k = sigma^k coeff of poly p)."""
    c0, h = (hi + lo) / 2.0, (hi - lo) / 2.0
    n = DEG + 1
    nodes = c0 + h * np.cos(np.pi * (2 * np.arange(n) + 1) / (2 * n))
    Ys = np.stack([np.stack(_scan_coeffs_of_s(sn, dt_val)) for sn in nodes])
    Vand = np.vander((nodes - c0) / h, n, increasing=True)
    coeffs = np.linalg.solve(Vand, Ys.reshape(n, -1)).reshape(n, 3, NV)
    pc = np.empty((3 * n, NV), np.float32)
    for p in range(3):
        for k in range(n):
            pc[4 * p + k] = coeffs[k, p]
    return pc, c0, h


def _emit(tc, o_ap, f_ap, pc_ap, v2_ap, id_ap, sc_mul, sc_sub):
    """Emit the per-core tile program body.
    pc_ap: [24, NV] f32r rows = [poly0 hi(4), poly0 lo(4), poly1 hi, ...].
    sigma = n2*rn4*sc_mul - sc_sub (immediates)."""
    from contextlib import ExitStack
    import concourse.bass as bass
    from concourse import mybir

    f32 = mybir.dt.float32
    f32r = mybir.dt.float32r
    MULT, ADD, SUB = (mybir.AluOpType.mult, mybir.AluOpType.add,
                      mybir.AluOpType.subtract)
    nc = tc.nc

    B = CFG["bufs"]
    with ExitStack() as ctx:
        singles = ctx.enter_context(tc.tile_pool(name="singles", bufs=1))
        pf = ctx.enter_context(tc.tile_pool(name="pf", bufs=B))
        pz = ctx.enter_context(tc.tile_pool(name="pz", bufs=B))
        pchi = ctx.enter_context(tc.tile_pool(name="pchi", bufs=B))
        px = ctx.enter_context(tc.tile_pool(name="px", bufs=B))
        pscr = ctx.enter_context(tc.tile_pool(name="pscr", bufs=B))
        ptiny = ctx.enter_context(tc.tile_pool(name="ptiny", bufs=B))
        pit = ctx.enter_context(tc.tile_pool(name="pit", bufs=B))
        if CFG["scan_split"]:
            psA0 = ctx.enter_context(tc.tile_pool(name="psA0", bufs=1, space="PSUM"))
            psA1 = ctx.enter_context(tc.tile_pool(name="psA1", bufs=1, space="PSUM"))
            psC0 = ctx.enter_context(tc.tile_pool(name="psC0", bufs=1, space="PSUM"))
            psC1 = ctx.enter_context(tc.tile_pool(name="psC1", bufs=1, space="PSUM"))
        else:
            psA = ctx.enter_context(tc.tile_pool(name="psA", bufs=1, space="PSUM"))
            psC = ctx.enter_context(tc.tile_pool(name="psC", bufs=1, space="PSUM"))
        psI = ctx.enter_context(tc.tile_pool(name="psI", bufs=1, space="PSUM"))
        psT = ctx.enter_context(tc.tile_pool(name="psT", bufs=2, space="PSUM"))

        tv2 = singles.tile([128, NV], f32)
        v2b = bass.AP(tensor=v2_ap.tensor, offset=v2_ap.offset,
                      ap=[[0, 128]] + [list(d) for d in v2_ap.ap[1:]])
        nc.sync.dma_start(tv2, v2b)
        if CFG["n4"] == "pool_act":
            tv4 = singles.tile([128, NV], f32)
            nc.gpsimd.tensor_mul(tv4, tv2, tv2)
        tpc = singles.tile([4, 6 * NV], f32r)
        nc.gpsimd.dma_start(tpc, pc_ap)
        tid = singles.tile([128, 128], f32)
        nc.gpsimd.dma_start(tid, id_ap)
        tpch = [tpc[:, (2 * p) * NV:(2 * p + 1) * NV] for p in range(3)]
        tpcl = [tpc[:, (2 * p + 1) * NV:(2 * p + 2) * NV] for p in range(3)]

        for w in range(CFG["warmup"]):
            pwarm = psT.tile([4, 128], f32, tag="ppwT")
            nc.tensor.transpose(pwarm, tid[:, 0:4], tid)

        for b in range(NBLK):
            rows = slice(b * 128, (b + 1) * 128)
            tf = pf.tile([128, NV], f32)
            nc.sync.dma_start(tf, f_ap[rows, :])

            # moments: n2 = sum f*v^2 ; n4 = sum (f*v^2)*v^2, each as one
            # fused DVE scalar_tensor_tensor with accum_out
            # (tensor_tensor_reduce aborts on this hardware/runtime)
            scr = pscr.tile([128, NV], f32, tag="scr")
            scr2 = pscr.tile([128, NV], f32, tag="scr2")
            n2 = ptiny.tile([128, 1], f32, tag="n2")
            n4 = ptiny.tile([128, 1], f32, tag="n4")
            nc.vector.scalar_tensor_tensor(scr, tf, 1.0, tv2, MULT, MULT,
                                           accum_out=n2)
            if CFG["n4"] == "pool_act":
                scrd = pscr.tile([128, NV], f32, tag="scrd")
                nc.gpsimd.tensor_mul(scr2, tf, tv4)
                nc.scalar.activation(scrd, scr2,
                                     mybir.ActivationFunctionType.Copy,
                                     bias=0.0, scale=1.0, accum_out=n4)
            else:
                nc.vector.scalar_tensor_tensor(scr2, scr, 1.0, tv2, MULT,
                                               MULT, accum_out=n4)

            # sigma and powers -> tpw [128,4] = [1, s, s^2, s^3]
            rn4 = ptiny.tile([128, 1], f32, tag="rn4")
            t1 = ptiny.tile([128, 1], f32, tag="t1")
            tpw = ptiny.tile([128, 4], f32, tag="tpw")
            nc.vector.reciprocal(rn4, n4)
            nc.vector.tensor_mul(t1, n2, rn4)
            nc.vector.memset(tpw[:, 0:1], 1.0)
            nc.vector.tensor_scalar(tpw[:, 1:2], t1, sc_mul, sc_sub, MULT, SUB)
            nc.vector.tensor_mul(tpw[:, 2:3], tpw[:, 1:2], tpw[:, 1:2])
            nc.vector.tensor_mul(tpw[:, 3:4], tpw[:, 2:3], tpw[:, 1:2])

            # lhsT: PE transpose -> [4,128] PSUM, ACT copy -> SBUF f32r
            ppwT = psT.tile([4, 128], f32, tag="ppwT")
            nc.tensor.transpose(ppwT, tpw, tid)
            tpwT = ptiny.tile([4, 128], f32r, tag="tpwT")
            nc.scalar.copy(tpwT, ppwT)

            # polys: per half, 2 accumulating f32r matmuls
            pI = psI.tile([128, NV], f32, tag="pI")
            HL = (slice(0, 512), slice(512, NV))
            def mm2(dst, p, cols, dcols):
                nc.tensor.matmul(dst[:, dcols], tpwT, tpch[p][:, cols],
                                 start=True, stop=False)
                nc.tensor.matmul(dst[:, dcols], tpwT, tpcl[p][:, cols],
                                 start=False, stop=True)

            tz = pz.tile([128, NV], f32)
            tchi = pchi.tile([128, NV], f32)
            if CFG["scan_split"]:
                pA0 = psA0.tile([128, 512], f32, tag="pA0")
                pA1 = psA1.tile([128, 512], f32, tag="pA1")
                pC0 = psC0.tile([128, 512], f32, tag="pC0")
                pC1 = psC1.tile([128, 512], f32, tag="pC1")
                for half, dst in ((0, pA0), (1, pA1)):
                    mm2(dst, 0, HL[half], slice(0, 512))
                for half, dst in ((0, pC0), (1, pC1)):
                    mm2(dst, 1, HL[half], slice(0, 512))
                for half in range(2):
                    mm2(pI, 2, HL[half], HL[half])
                # scan1 fwd, chained halves
                nc.vector.tensor_tensor_scan(tz[:, HL[0]], pA0, tf[:, HL[0]],
                                             0.0, MULT, ADD)
                nc.vector.tensor_tensor_scan(tz[:, HL[1]], pA1, tf[:, HL[1]],
                                             tz[:, 511:512], MULT, ADD)
                # scan2 bwd, chained reversed halves
                nc.vector.tensor_tensor_scan(tchi[:, HL[1]][:, ::-1],
                                             pC1[:, ::-1],
                                             tz[:, HL[1]][:, ::-1],
                                             0.0, MULT, ADD)
                nc.vector.tensor_tensor_scan(tchi[:, HL[0]][:, ::-1],
                                             pC0[:, ::-1],
                                             tz[:, HL[0]][:, ::-1],
                                             tchi[:, 512:513], MULT, ADD)
            else:
                pA = psA.tile([128, NV], f32, tag="pA")
                pC = psC.tile([128, NV], f32, tag="pC")
                for half in range(2):
                    mm2(pA, 0, HL[half], HL[half])
                for half in range(2):
                    mm2(pC, 1, HL[half], HL[half])
                for half in range(2):
                    mm2(pI, 2, HL[half], HL[half])
                nc.vector.tensor_tensor_scan(tz, pA, tf, 0.0, MULT, ADD)
                nc.vector.tensor_tensor_scan(tchi[:, ::-1], pC[:, ::-1],
                                             tz[:, ::-1], 0.0, MULT, ADD)

            # x = it * chi
            tx = px.tile([128, NV], f32)
            if CFG["xmul"] == "pool":
                tit = pit.tile([128, NV], f32)
                nc.scalar.copy(tit, pI)
                nc.gpsimd.tensor_mul(tx, tit, tchi)
            elif CFG["xmul"] == "split":
                tit = pit.tile([128, NV], f32)
                nc.scalar.copy(tit[:, 512:], pI[:, 512:])
                nc.vector.tensor_mul(tx[:, :512], pI[:, :512], tchi[:, :512])
                nc.gpsimd.tensor_mul(tx[:, 512:], tit[:, 512:], tchi[:, 512:])
            else:
                nc.vector.tensor_mul(tx, pI, tchi)

            nc.scalar.dma_start(o_ap[rows, :], tx)


def _build_program(sc_mul, sc_sub):
    """Standalone Bacc program for one core: f [ROWS,NV] -> o [ROWS,NV]."""
    import concourse.bacc as bacc
    import concourse.tile as tile
    from concourse import mybir

    f32 = mybir.dt.float32
    f32r = mybir.dt.float32r
    nc = bacc.Bacc("TRN2", target_bir_lowering=False, debug=False,
                   num_devices=N_CORES)
    f_ap = nc.dram_tensor("f_in", [ROWS, NV], f32, kind="ExternalInput").ap()
    pc_ap = nc.dram_tensor("pcoef", [4, 6 * NV], f32r, kind="ExternalInput").ap()
    v2_ap = nc.dram_tensor("v2row", [1, NV], f32, kind="ExternalInput").ap()
    id_ap = nc.dram_tensor("ident", [128, 128], f32, kind="ExternalInput").ap()
    o_ap = nc.dram_tensor("o", [ROWS, NV], f32, kind="ExternalOutput").ap()
    with tile.TileContext(nc) as tc:
        _emit(tc, o_ap, f_ap, pc_ap, v2_ap, id_ap, sc_mul, sc_sub)
    nc.compile()
    return nc


def _pack_pc(pc):
    """Split fitted coeffs into tf32 hi/lo, pack [4, 6*NV] (f32r bits).
    Column block (2p+h)*NV holds poly p hi (h=0) / lo (h=1), row = degree."""
    hi = _tf32_rne(pc)
    lo = _tf32_rne(pc - hi)
    out = np.empty((4, 6 * NV), np.float32)
    for p in range(3):
        for k in range(4):
            out[k, (2 * p) * NV:(2 * p + 1) * NV] = hi[4 * p + k]
            out[k, (2 * p + 1) * NV:(2 * p + 2) * NV] = lo[4 * p + k]
    return out


def kernel(**inputs):
    f0x = np.ascontiguousarray(np.asarray(inputs["f0x"], dtype=np.float32))
    dt_val = float(np.asarray(inputs["dt"], dtype=np.float32))
    assert f0x.shape == (NX, NV)

    # host-side calibration of the fit interval (all f0x math runs on HW)
    fd = f0x.astype(np.float64)
    s_rows = 3.0 * DV * (fd @ (V**2)) / (fd @ (V**4))
    lo = s_rows.min() * 0.995
    hi = s_rows.max() * 1.005
    pc, c0, h = _fit_pc(dt_val, lo, hi)
    sc_mul = float(3.0 * DV / h)
    sc_sub = float(c0 / h)

    key = (round(sc_mul, 12), round(sc_sub, 12))
    if key not in _prog_cache:
        _prog_cache.clear()
        _prog_cache[key] = _build_program(sc_mul, sc_sub)
    nc = _prog_cache[key]

    pcoef = _pack_pc(pc)
    v2row = (V.astype(np.float32) ** 2).reshape(1, NV)
    ident = np.eye(128, dtype=np.float32)
    in_maps = []
    for r in range(N_CORES):
        in_maps.append({
            "f_in": np.ascontiguousarray(f0x[r * ROWS:(r + 1) * ROWS]),
            "pcoef": pcoef,
            "v2row": v2row,
            "ident": ident,
        })

    from concourse.bass_utils import run_bass_kernel_spmd
    res = run_bass_kernel_spmd(nc, in_maps, core_ids=list(range(N_CORES)))
    global _last_results
    _last_results = res
    out = np.concatenate([res.results[r]["o"] for r in range(N_CORES)], axis=0)
    return out.astype(np.float32)


_last_results = None



# revision 5
# speedup vs baseline: 1.3058x; 1.3058x over previous
"""Trainium2 Bass kernel for nn_F0Collisions (Chang-Cooper implicit collision step).

Approach: each row's tridiagonal system depends on the row only through
s = 2*beta*dv (beta from two moments of f0x). The Thomas-solve scan
coefficients
    At_j = -l_j / t_{j-1}   (forward:  z_j   = At_j z_{j-1} + f_j)
    ch_j = -u_j / t_{j+1}   (backward: chi_j = ch_j chi_{j+1} + z_j)
    it_j =  1 / t_j         (final:    x_j   = it_j * chi_j)
(t = LU pivots) are analytic in s; a degree-3 Chebyshev fit per j with the
coefficients split into tf32 hi/lo halves gives ~6e-4 end-to-end error.

Engine assignment per 128-row block (DVE is the serial wall — the two
scans can run nowhere else):
  DVE : 2 stride-4 subsampled moment reductions + reciprocal + 2 scans
  Pool: sigma/power tiny chain + final x = it*chi multiply
  ACT : PSUM->SBUF copies of the 3 coefficient fields (scans read SBUF)
  PE  : powers transpose + 6 f32r matmuls (hi+lo stacked on the contract
        dim: lhsT [8,128] = powers twice, rhs [8,512] = hi rows 0-3 over
        lo rows 4-7 -> one matmul per poly-half)
All moment work for all 4 blocks is emitted before any scans so the DVE
scan stretch runs uninterrupted.

8 cores, data-parallel over rows: 512 rows/core.
"""
import numpy as np

NX, NV = 4096, 1024
VMAX, NUEE = 8.0, 1.0
DV = VMAX / NV
V = (np.arange(NV, dtype=np.float64) + 0.5) * DV
V_EDGE = np.arange(NV + 1, dtype=np.float64) * DV
N_CORES = 8
ROWS = NX // N_CORES          # 512 rows per core
NBLK = ROWS // 128            # 4 blocks of 128 rows
DEG = 3                       # Chebyshev degree in sigma
MSTRIDE = 4                   # moment subsample stride
MS = NV // MSTRIDE            # subsampled length

_prog_cache = {}


def _tf32_rne(x):
    xi = np.asarray(x, np.float32).view(np.uint32)
    r = (xi.astype(np.uint64) + 0x1000 + ((xi >> 13) & 1)).astype(np.uint64)
    return (r & np.uint64(0xFFFFE000)).astype(np.uint32).view(np.float32)


def _cc_delta(w):
    small = np.abs(w) < 1e-8
    ws = np.where(small, 1.0, w)
    return np.where(small, 0.5, 1.0 / ws - 1.0 / np.expm1(ws))


def _scan_coeffs_of_s(s, dt_val):
    """Exact At, ch, it for scalar s = 2*beta*DV (float64)."""
    ve = V_EDGE
    rD = 1.0 / s                       # D/DV = 1/(2 beta DV)
    delta = _cc_delta(s * ve)
    a = ve * delta - rD
    b = ve * (1.0 - delta) + rD
    a[0] = b[0] = a[NV] = b[NV] = 0.0
    coef = dt_val * (NUEE / V**2) / DV
    l = coef * a[:-1]
    d = 1.0 - coef * (a[1:] - b[:-1])
    u = -coef * b[1:]
    t = np.empty(NV)
    t[0] = d[0]
    for j in range(1, NV):
        t[j] = d[j] - l[j] * u[j - 1] / t[j - 1]
    At = np.zeros(NV); At[1:] = -l[1:] / t[:-1]
    it = 1.0 / t
    ch = np.zeros(NV); ch[:-1] = -u[:-1] / t[1:]
    return At, ch, it


def _fit_pc(dt_val, lo, hi):
    """Degree-DEG Chebyshev-node fit in sigma=(s-c0)/h for At, ch, it.
    Returns coeffs [DEG+1, 3, NV] f64 plus (c0, h)."""
    c0, h = (hi + lo) / 2.0, (hi - lo) / 2.0
    n = DEG + 1
    nodes = c0 + h * np.cos(np.pi * (2 * np.arange(n) + 1) / (2 * n))
    Ys = np.stack([np.stack(_scan_coeffs_of_s(sn, dt_val)) for sn in nodes])
    Vand = np.vander((nodes - c0) / h, n, increasing=True)
    coeffs = np.linalg.solve(Vand, Ys.reshape(n, -1)).reshape(n, 3, NV)
    return coeffs, c0, h


def _pack_pc(coeffs):
    """Pack hi/lo tf32 halves stacked on the contract dim: [8, 3*NV].
    Row k (0-3) = sigma^k hi coeff, row 4+k = sigma^k lo coeff; column
    block p*NV holds poly p."""
    n = DEG + 1
    out = np.empty((2 * n, 3 * NV), np.float32)
    for p in range(3):
        for k in range(n):
            c = coeffs[k, p].astype(np.float32)
            hi = _tf32_rne(c)
            lo = _tf32_rne(c - hi)
            out[k, p * NV:(p + 1) * NV] = hi
            out[n + k, p * NV:(p + 1) * NV] = lo
    return out


def _emit(tc, o_ap, f_ap, pc_ap, v2s_ap, v4s_ap, id_ap, sc_mul, sc_sub):
    """Emit the per-core tile program body."""
    from contextlib import ExitStack
    import concourse.bass as bass
    from concourse import mybir

    f32 = mybir.dt.float32
    f32r = mybir.dt.float32r
    MULT, ADD, SUB = (mybir.AluOpType.mult, mybir.AluOpType.add,
                      mybir.AluOpType.subtract)
    nc = tc.nc

    with ExitStack() as ctx:
        singles = ctx.enter_context(tc.tile_pool(name="singles", bufs=1))
        pf = ctx.enter_context(tc.tile_pool(name="pf", bufs=1))
        pco = ctx.enter_context(tc.tile_pool(name="pco", bufs=1))
        pz = ctx.enter_context(tc.tile_pool(name="pz", bufs=2))
        pchi = ctx.enter_context(tc.tile_pool(name="pchi", bufs=2))
        px = ctx.enter_context(tc.tile_pool(name="px", bufs=2))
        ptiny = ctx.enter_context(tc.tile_pool(name="ptiny", bufs=1))
        psA = ctx.enter_context(tc.tile_pool(name="psA", bufs=1, space="PSUM"))
        psC = ctx.enter_context(tc.tile_pool(name="psC", bufs=1, space="PSUM"))
        psI = ctx.enter_context(tc.tile_pool(name="psI", bufs=1, space="PSUM"))
        psT = ctx.enter_context(tc.tile_pool(name="psT", bufs=2, space="PSUM"))

        # constant tables
        tv2s = singles.tile([128, MS], f32)
        tv4s = singles.tile([128, MS], f32)
        for t_ap, src in ((tv2s, v2s_ap), (tv4s, v4s_ap)):
            b = bass.AP(tensor=src.tensor, offset=src.offset,
                        ap=[[0, 128]] + [list(d) for d in src.ap[1:]])
            nc.sync.dma_start(t_ap, b)
        tpc = singles.tile([2 * (DEG + 1), 3 * NV], f32r)
        nc.gpsimd.dma_start(tpc, pc_ap)
        tid = singles.tile([128, 128], f32)
        nc.gpsimd.dma_start(tid, id_ap)

        # f DMAs for all blocks up front (two queues)
        tfs = []
        for b in range(NBLK):
            rows = slice(b * 128, (b + 1) * 128)
            tf = pf.tile([128, NV], f32, tag=f"tf{b}")
            nc.sync.dma_start(tf, f_ap[rows, :])
            tfs.append(tf)

        # phase 1: moments + sigma powers + coefficient fields per block
        coef_tiles = []
        for b in range(NBLK):
            tf = tfs[b]
            # stride-4 subsampled moments: n2 = sum f*v^2, n4 = sum f*v^4
            fsub = bass.AP(tensor=tf.tensor, offset=tf.offset,
                           ap=[list(tf.ap[0]), [MSTRIDE, MS]])
            scr = ptiny.tile([128, MS], f32, tag=f"scr{b}")
            scr2 = ptiny.tile([128, MS], f32, tag=f"scr2{b}")
            n2 = ptiny.tile([128, 1], f32, tag=f"n2{b}")
            n4 = ptiny.tile([128, 1], f32, tag=f"n4{b}")
            nc.vector.scalar_tensor_tensor(scr, fsub, 1.0, tv2s, MULT, MULT,
                                           accum_out=n2)
            nc.vector.scalar_tensor_tensor(scr2, fsub, 1.0, tv4s, MULT, MULT,
                                           accum_out=n4)
            rn4 = ptiny.tile([128, 1], f32, tag=f"rn4{b}")
            nc.vector.reciprocal(rn4, n4)

            # sigma and powers on Pool -> tpw [128,4] = [1, s, s^2, s^3]
            t1 = ptiny.tile([128, 1], f32, tag=f"t1{b}")
            tpw = ptiny.tile([128, 8], f32, tag=f"tpw{b}")
            nc.gpsimd.tensor_tensor(t1, n2, rn4, MULT)
            nc.gpsimd.memset(tpw[:, 0:1], 1.0)
            nc.gpsimd.tensor_scalar(tpw[:, 1:2], t1, sc_mul, sc_sub, MULT, SUB)
            nc.gpsimd.tensor_tensor(tpw[:, 2:3], tpw[:, 1:2], tpw[:, 1:2], MULT)
            nc.gpsimd.tensor_tensor(tpw[:, 3:4], tpw[:, 2:3], tpw[:, 1:2], MULT)
            # duplicate the powers into cols 4-7 (hi/lo stacked contract dim)
            nc.gpsimd.tensor_scalar(tpw[:, 4:8], tpw[:, 0:4], 1.0, 0.0,
                                    MULT, ADD)

            # lhsT: PE transpose -> [8,128] PSUM, ACT copy -> SBUF f32r
            ppwT = psT.tile([8, 128], f32, tag="ppwT")
            nc.tensor.transpose(ppwT, tpw, tid)
            tpwT = ptiny.tile([8, 128], f32r, tag=f"tpwT{b}")
            nc.scalar.copy(tpwT, ppwT)

            # coefficient fields: 1 matmul per poly-half (hi+lo stacked)
            pA = psA.tile([128, NV], f32, tag="pA")
            pC = psC.tile([128, NV], f32, tag="pC")
            pI = psI.tile([128, NV], f32, tag="pI")
            tAt = pco.tile([128, NV], f32, tag=f"tAt{b}")
            tch = pco.tile([128, NV], f32, tag=f"tch{b}")
            tit = pco.tile([128, NV], f32, tag=f"tit{b}")
            for p, (dst, sb) in enumerate(((pA, tAt), (pC, tch), (pI, tit))):
                for h in range(2):
                    cols = slice(p * NV + h * 512, p * NV + (h + 1) * 512)
                    nc.tensor.matmul(dst[:, h * 512:(h + 1) * 512], tpwT,
                                     tpc[:, cols], start=True, stop=True)
                nc.scalar.copy(sb, dst)
            coef_tiles.append((tAt, tch, tit))

        # phase 2: scans + final multiply per block
        for b in range(NBLK):
            tf = tfs[b]
            tAt, tch, tit = coef_tiles[b]
            tz = pz.tile([128, NV], f32)
            tchi = pchi.tile([128, NV], f32)
            nc.vector.tensor_tensor_scan(tz, tAt, tf, 0.0, MULT, ADD)
            nc.vector.tensor_tensor_scan(tchi[:, ::-1], tch[:, ::-1],
                                         tz[:, ::-1], 0.0, MULT, ADD)
            tx = px.tile([128, NV], f32)
            nc.gpsimd.tensor_tensor(tx, tit, tchi, MULT)
            rows = slice(b * 128, (b + 1) * 128)
            nc.scalar.dma_start(o_ap[rows, :], tx)


def _build_program(sc_mul, sc_sub):
    """Standalone Bacc program for one core: f [ROWS,NV] -> o [ROWS,NV]."""
    import concourse.bacc as bacc
    import concourse.tile as tile
    from concourse import mybir

    f32 = mybir.dt.float32
    f32r = mybir.dt.float32r
    nc = bacc.Bacc("TRN2", target_bir_lowering=False, debug=False,
                   num_devices=N_CORES)
    f_ap = nc.dram_tensor("f_in", [ROWS, NV], f32, kind="ExternalInput").ap()
    pc_ap = nc.dram_tensor("pcoef", [2 * (DEG + 1), 3 * NV], f32r,
                           kind="ExternalInput").ap()
    v2s_ap = nc.dram_tensor("v2sub", [1, MS], f32, kind="ExternalInput").ap()
    v4s_ap = nc.dram_tensor("v4sub", [1, MS], f32, kind="ExternalInput").ap()
    id_ap = nc.dram_tensor("ident", [128, 128], f32, kind="ExternalInput").ap()
    o_ap = nc.dram_tensor("o", [ROWS, NV], f32, kind="ExternalOutput").ap()
    with tile.TileContext(nc) as tc:
        _emit(tc, o_ap, f_ap, pc_ap, v2s_ap, v4s_ap, id_ap, sc_mul, sc_sub)
    nc.compile()
    return nc


def kernel(**inputs):
    f0x = np.ascontiguousarray(np.asarray(inputs["f0x"], dtype=np.float32))
    dt_val = float(np.asarray(inputs["dt"], dtype=np.float32))
    assert f0x.shape == (NX, NV)

    # host-side calibration of the fit interval from the same stride-4
    # subsampled moments the HW computes (all f0x field math runs on HW)
    fd = f0x.astype(np.float64)[:, ::MSTRIDE]
    v2s = (V**2)[::MSTRIDE]
    v4s = (V**4)[::MSTRIDE]
    s_rows = 3.0 * DV * (fd @ v2s) / (fd @ v4s)
    lo = s_rows.min() * 0.995
    hi = s_rows.max() * 1.005
    coeffs, c0, h = _fit_pc(dt_val, lo, hi)
    sc_mul = float(3.0 * DV / h)
    sc_sub = float(c0 / h)

    key = (round(sc_mul, 12), round(sc_sub, 12))
    if key not in _prog_cache:
        _prog_cache.clear()
        _prog_cache[key] = _build_program(sc_mul, sc_sub)
    nc = _prog_cache[key]

    pcoef = _pack_pc(coeffs)
    v2row = v2s.astype(np.float32).reshape(1, MS)
    v4row = v4s.astype(np.float32).reshape(1, MS)
    ident = np.eye(128, dtype=np.float32)
    in_maps = []
    for r in range(N_CORES):
        in_maps.append({
            "f_in": np.ascontiguousarray(f0x[r * ROWS:(r + 1) * ROWS]),
            "pcoef": pcoef,
            "v2sub": v2row,
            "v4sub": v4row,
            "ident": ident,
        })

    from concourse.bass_utils import run_bass_kernel_spmd
    res = run_bass_kernel_spmd(nc, in_maps, core_ids=list(range(N_CORES)))
    global _last_results
    _last_results = res
    out = np.concatenate([res.results[r]["o"] for r in range(N_CORES)], axis=0)
    return out.astype(np.float32)


_last_results = None
